# revision 7
# baseline (speedup 1.0000x reference)
"""Memory-Network kernel for 8 Trainium2 NeuronCores.

Data-parallel: batch B=128 is split 16-per-core; each core processes its
160 (b, r) sequences end-to-end (embedding gather, q/f LSTMs, attention,
FC) with no collectives. Weights are replicated; all layout prep
(transposes, gate permutation, dtype casts, padding) happens on host.

Device layout convention: everything feature-major [feature, token] so
the LSTM recurrence's matmuls keep gates/hidden on the partition dim.

Per-core embedding rows are host-deduplicated into one compact table
(<= 9728 rows, int16-indexable) so each token needs a single gather.
The two LSTMs are interleaved 1q:2f so both finish together and each
stream's elementwise chain hides under the other's matmuls.

LSTM + attention matmuls run in fp8 (e4m3, DoubleRow: K=256 per
instruction). A x8 scale is folded into the host-side tables (emb x8,
W_hh x8, W2 x8; W_ih true) so gate PSUM holds 8x the true value and the
activation's free `scale=1/8` descales. The W1/img projection stays
bf16: its output feeds the final sum unattenuated and fp8 there costs
~10x in max error.
"""

import sys

for _p in ("/opt/trn_rl_repo", "/root/.axon_site/_ro/trn_rl_repo"):
    if _p not in sys.path:
        sys.path.insert(0, _p)

import numpy as np
import ml_dtypes

import concourse.bass as bass
import concourse.mybir as mybir
import concourse.tile as tile
from concourse import bacc
from concourse.bass_utils import run_bass_kernel_spmd
from concourse.masks import make_identity

BF16 = mybir.dt.bfloat16
F32 = mybir.dt.float32
F8 = mybir.dt.float8e4
I16 = mybir.dt.int16
DR = mybir.MatmulPerfMode.DoubleRow

NP_BF16 = ml_dtypes.bfloat16
NP_F8 = ml_dtypes.float8_e4m3

VOCAB, EMB, HID, IMG = 50000, 300, 512, 4096
B, R, LQ, LH = 128, 10, 20, 40
N_CORES = 8
BS = B // N_CORES          # 16 batch items per core
S = BS * R                 # 160 sequences per core
EPAD = 384                 # embedding row padded to 3x128 for K-chunking
KX2 = 64                   # rows of x-chunk 2 (features 256..299 + bias,
                           # zero-padded to the exact 64-row PE tile size)
G4 = 4 * HID               # 2048 gate rows
NEG = -1.0e30
NU_PAD = 9728              # compact per-core embedding table rows (>= uniques)
SC = 8.0                   # fp8 scale folded into emb/Whh/W2 tables
DSC = 1.0 / SC

_STATE = None


def _gate_perm():
    # m-tile m = 4*j + pos: hidden chunk j's gates in order (i, f, o, g) so
    # the three sigmoids sit in adjacent PSUM banks (one batched ACT op)
    return np.concatenate(
        [np.arange(g * HID + j * 128, g * HID + (j + 1) * 128)
         for j in range(4) for g in (0, 1, 3, 2)]
    )


def _build_program():
    nc = bacc.Bacc(num_swdge_queues=4)

    dt_in = {}

    def din(name, shape, dtype):
        dt_in[name] = nc.dram_tensor(name, list(shape), dtype, kind="ExternalInput")
        return dt_in[name]

    embC_d = din("embC", [NU_PAD, EPAD], BF16)
    idxq_d = din("idxq", [128, LQ * S // 16], I16)   # [128, 200]
    idxf_d = din("idxf", [128, LH * S // 16], I16)   # [128, 400]
    wqx_d = din("wqx", [EPAD, G4], F8)
    wqh_d = din("wqh", [HID, G4], F8)
    wfx_d = din("wfx", [EPAD, G4], F8)
    wfh_d = din("wfh", [HID, G4], F8)
    w1i_d = din("w1i", [IMG, HID], BF16)
    w1h_d = din("w1h", [HID, HID], BF16)
    b1_d = din("b1", [128, 4], F32)
    w2_d = din("w2", [HID, HID], F8)
    b2_d = din("b2", [128, 4], F32)
    img_d = din("imgrep", [IMG, S], BF16)
    mask_d = din("mask", [S, S], F32)
    out_d = nc.dram_tensor("out", [HID, S], F32, kind="ExternalOutput")

    with tile.TileContext(nc) as tc:
        with (
            tc.tile_pool(name="consts", bufs=1) as cp,
            tc.tile_pool(name="gather", bufs=4) as gp,
            tc.tile_pool(name="hstate", bufs=12) as hp,
            tc.tile_pool(name="cstate", bufs=8) as cpool,
            tc.tile_pool(name="ew", bufs=16) as ew,
            tc.tile_pool(name="w1s", bufs=2) as w1p,
            tc.tile_pool(name="outp", bufs=3) as op,
            tc.tile_pool(name="ps", bufs=2, space="PSUM") as ps,
        ):
            # ---------- phase 0: index/weight DMAs, gathers ----------
            # sync queue: q-stream + late-use consts; scalar queue: f-stream.
            idxq_sb = cp.tile(list(idxq_d.shape), I16, name="idxq_sb", tag="idxq")
            nc.sync.dma_start(idxq_sb[:], idxq_d.ap()[:])
            idxf_sb = cp.tile(list(idxf_d.shape), I16, name="idxf_sb", tag="idxf")
            nc.scalar.dma_start(idxf_sb[:], idxf_d.ap()[:])

            wq_sb = cp.tile([128, 7, G4], F8, name="wq_sb", tag="wq")
            nc.sync.dma_start(
                wq_sb[:, 0:3, :],
                wqx_d.ap().rearrange("(k p) m -> p k m", p=128))
            nc.sync.dma_start(
                wq_sb[:, 3:7, :],
                wqh_d.ap().rearrange("(k p) m -> p k m", p=128))
            wf_sb = cp.tile([128, 7, G4], F8, name="wf_sb", tag="wf")
            nc.scalar.dma_start(
                wf_sb[:, 0:3, :],
                wfx_d.ap().rearrange("(k p) m -> p k m", p=128))
            nc.scalar.dma_start(
                wf_sb[:, 3:7, :],
                wfh_d.ap().rearrange("(k p) m -> p k m", p=128))

            # late-use consts; sync engine runs ahead, transfers overlap LSTM
            w1h_sb = cp.tile([128, 4, HID], BF16, name="w1h_sb", tag="w1h")
            nc.sync.dma_start(
                w1h_sb[:], w1h_d.ap().rearrange("(k p) m -> p k m", p=128))
            w2_sb = cp.tile([128, 4, HID], F8, name="w2_sb", tag="w2")
            nc.sync.dma_start(
                w2_sb[:], w2_d.ap().rearrange("(k p) m -> p k m", p=128))
            b1_sb = cp.tile([128, 4], F32, name="b1_sb", tag="b1")
            nc.sync.dma_start(b1_sb[:], b1_d.ap()[:])
            b2_sb = cp.tile([128, 4], F32, name="b2_sb", tag="b2")
            nc.sync.dma_start(b2_sb[:], b2_d.ap()[:])
            img_sb = cp.tile([128, IMG // 128, S], BF16, name="img_sb", tag="img")
            nc.sync.dma_start(
                img_sb[:], img_d.ap().rearrange("(k p) m -> p k m", p=128))
            mask_sb = cp.tile([128, 2, S], F32, name="mask_sb", tag="mask")
            nc.sync.dma_start(
                mask_sb[:, 0, :], mask_d.ap()[0:128, :])
            nc.sync.dma_start(
                mask_sb[0:S - 128, 1, :], mask_d.ap()[128:S, :])

            ident = cp.tile([128, 128], BF16, name="ident", tag="ident")
            make_identity(nc, ident[:])

            # gathered embeddings: bf16 rows land in a rotating pool, then
            # cast to persistent fp8 tiles, feature-major [128, 3, GRP]
            GRP = 4 * S
            NIC = GRP // 16               # idx columns per group
            eq = [cp.tile([128, 3, GRP], F8, name=f"eq_{g}", tag=f"eq_{g}")
                  for g in range(LQ // 4)]
            ef = [cp.tile([128, 3, GRP], F8, name=f"ef_{g}", tag=f"ef_{g}")
                  for g in range(LH // 4)]

            qn_ctr = [0]

            def gather_group(idx_sb, g, dst):
                qn = qn_ctr[0] % 4
                qn_ctr[0] += 1
                gb = gp.tile([128, 3, GRP], BF16, name="gb", tag="gb")
                nc.gpsimd.dma_gather(
                    out_ap=gb[:],
                    in_ap=embC_d.ap()[:],
                    idxs_ap=idx_sb[:, g * NIC:(g + 1) * NIC],
                    num_idxs=GRP,
                    num_idxs_reg=GRP,
                    elem_size=EPAD,
                    transpose=True,
                    queue_num=qn,
                )
                nc.vector.tensor_copy(dst[g][:], gb[:])

            # feed order matches 1q:2f consumption: window w uses eq[w//4],
            # ef[w//2]
            for g in range(LQ // 4):
                gather_group(idxq_sb, g, eq)
                gather_group(idxf_sb, 2 * g, ef)
                gather_group(idxf_sb, 2 * g + 1, ef)

            # ---------- LSTM recurrence ----------
            # One step of one stream, fp8 DoubleRow: x chunks (0,1) + h
            # chunks (0,1),(2,3) as K=256 DR ops, x chunk 2 as a K=64 op.
            # PSUM gates hold 8x the true preactivation (emb/Whh tables are
            # x8); the sigmoid/tanh descale via scale=1/8. Gate biases ride
            # the x-matmul (embedding col 300 is 8.0; weight row 300 = bias).
            # Gates per hidden chunk j land in one 4-bank PSUM tile in order
            # (i, f, o, g) so the three sigmoids are one strided ACT op.
            def lstm_step(state, t, e_chunks, w_sb, label):
                h, c_st = state
                ec = e_chunks[t // 4]
                co = (t % 4) * S
                x01 = ec[:, 0:2, co:co + S]
                x2 = ec[0:KX2, 2, co:co + S]
                new_h = hp.tile([128, 4, S], F8, name="hn", tag="h", bufs=4)
                new_c = cpool.tile([128, 4, S], F32, name="cn", tag="c", bufs=4)
                for j in range(4):
                    pg = ps.tile([128, 4, S], F32, name=f"pg{label}", tag="pg",
                                 padded_shape=[128, 4, 512])
                    for g in range(4):
                        m = 4 * j + g
                        mc = slice(m * 128, (m + 1) * 128)
                        nc.tensor.matmul(
                            pg[:, g, :], lhsT=w_sb[:, 0:2, mc], rhs=x01,
                            start=True, stop=False, perf_mode=DR)
                        nc.tensor.matmul(
                            pg[:, g, :], lhsT=w_sb[0:KX2, 2, mc], rhs=x2,
                            start=False, stop=(t == 0))
                        if t > 0:
                            nc.tensor.matmul(
                                pg[:, g, :], lhsT=w_sb[:, 3:5, mc],
                                rhs=h[:, 0:2, :],
                                start=False, stop=False, perf_mode=DR)
                            nc.tensor.matmul(
                                pg[:, g, :], lhsT=w_sb[:, 5:7, mc],
                                rhs=h[:, 2:4, :],
                                start=False, stop=True, perf_mode=DR)
                    sig = ew.tile([128, 3, S], F32, name="sig", tag="sig", bufs=6)
                    nc.scalar.activation(
                        sig[:], pg[:, 0:3, :],
                        mybir.ActivationFunctionType.Sigmoid, scale=DSC)
                    tg = ew.tile([128, S], F32, name="tg", tag="ew")
                    nc.scalar.activation(
                        tg[:], pg[:, 3, :], mybir.ActivationFunctionType.Tanh,
                        scale=DSC)
                    cn = new_c[:, j, :]
                    if t == 0:
                        nc.vector.tensor_mul(cn, sig[:, 0, :], tg[:])
                    else:
                        m1 = ew.tile([128, S], F32, name="m1", tag="ew")
                        nc.vector.tensor_mul(m1[:], sig[:, 1, :], c_st[:, j, :])
                        m2 = ew.tile([128, S], F32, name="m2", tag="ew")
                        nc.vector.tensor_mul(m2[:], sig[:, 0, :], tg[:])
                        nc.vector.tensor_add(cn, m1[:], m2[:])
                    tc_ = ew.tile([128, S], F32, name="tc", tag="ew")
                    nc.scalar.activation(
                        tc_[:], cn, mybir.ActivationFunctionType.Tanh)
                    nc.vector.tensor_mul(new_h[:, j, :], sig[:, 2, :], tc_[:])
                return (new_h, new_c)

            # interleave 1q:2f so the two streams hide each other's
            # elementwise chains and finish together (no solo tail)
            st_q = (None, None)
            st_f = (None, None)
            for w in range(LQ):
                st_q = lstm_step(st_q, w, eq, wq_sb, "q")
                st_f = lstm_step(st_f, 2 * w, ef, wf_sb, "f")
                st_f = lstm_step(st_f, 2 * w + 1, ef, wf_sb, "f")
            hq_t = st_q[0]
            hf_t = st_f[0]
            hf = [hf_t[:, j, :] for j in range(4)]

            # bf16 copy of hq for the bf16 W1 projection
            hq_bf = cp.tile([128, 4, S], BF16, name="hq_bf", tag="hq_bf")
            nc.vector.tensor_copy(hq_bf[:], hq_t[:])

            # ---------- query = tanh([img, hq] @ W1.T + b1) ----------
            pq = ps.tile([128, 4, S], F32, name="pq", tag="pg",
                         padded_shape=[128, 4, 512])

            def qslice(m):
                return pq[:, m, :]

            n_im_blk = IMG // 256  # 16 streamed lhsT blocks of 2 k-chunks
            for bI in range(n_im_blk):
                w1c = w1p.tile([128, 2, HID], BF16, name="w1c", tag="w1c")
                nc.sync.dma_start(
                    w1c[:],
                    w1i_d.ap()[bI * 256:(bI + 1) * 256, :].rearrange(
                        "(k p) m -> p k m", p=128))
                for k8 in range(2):
                    ki = bI * 2 + k8
                    for m in range(4):
                        nc.tensor.matmul(
                            qslice(m),
                            lhsT=w1c[:, k8, m * 128:(m + 1) * 128],
                            rhs=img_sb[:, ki, :],
                            start=(ki == 0),
                            stop=False,
                        )
            for k in range(4):
                for m in range(4):
                    nc.tensor.matmul(
                        qslice(m),
                        lhsT=w1h_sb[:, k, m * 128:(m + 1) * 128],
                        rhs=hq_bf[:, k, :],
                        start=False,
                        stop=(k == 3),
                    )
            qt_f = []
            qt_b = cp.tile([128, 4, S], F8, name="qt_b", tag="qt_b")
            for m in range(4):
                qf = cp.tile([128, S], F32, name=f"qtf{m}", tag=f"qtf{m}")
                nc.scalar.activation(
                    qf[:], qslice(m), mybir.ActivationFunctionType.Tanh,
                    bias=b1_sb[:, m:m + 1])
                nc.vector.tensor_copy(qt_b[:, m, :], qf[:])
                qt_f.append(qf)

            # ---------- attention ----------
            # scores[n, n'] = sum_h Q[h, n] hf[h, n']  (2 partition tiles of n)
            sct = ps.tile([128, 4, S], F32, name="sct", tag="pg",
                          padded_shape=[128, 4, 512])
            sc0, sc1 = sct[:, 0, :], sct[0:S - 128, 1, :]
            for kk in range(2):
                nc.tensor.matmul(sc0, lhsT=qt_b[:, 2 * kk:2 * kk + 2, 0:128],
                                 rhs=hf_t[:, 2 * kk:2 * kk + 2, :],
                                 start=(kk == 0), stop=(kk == 1), perf_mode=DR)
            for kk in range(2):
                nc.tensor.matmul(sc1, lhsT=qt_b[:, 2 * kk:2 * kk + 2, 128:S],
                                 rhs=hf_t[:, 2 * kk:2 * kk + 2, :],
                                 start=(kk == 0), stop=(kk == 1), perf_mode=DR)

            a_bf = []  # attention weights, 2 partition tiles [*, S] fp8
            for ti, (scp, npart) in enumerate([(sc0, 128), (sc1, S - 128)]):
                sm = ew.tile([128, S], F32, name="sm", tag="ew")
                nc.vector.tensor_add(sm[:npart], scp, mask_sb[:npart, ti, :])
                nmx = ew.tile([128, 1], F32, name="nmx", tag="red", bufs=4)
                nc.vector.tensor_reduce(
                    nmx[:npart], sm[:npart], mybir.AxisListType.X,
                    mybir.AluOpType.max, negate=True)
                ex = ew.tile([128, S], F32, name="ex", tag="ew")
                nc.scalar.activation(
                    ex[:npart], sm[:npart], mybir.ActivationFunctionType.Exp,
                    bias=nmx[:npart])
                ssum = ew.tile([128, 1], F32, name="ssum", tag="red", bufs=4)
                nc.vector.tensor_reduce(
                    ssum[:npart], ex[:npart], mybir.AxisListType.X,
                    mybir.AluOpType.add)
                rs = ew.tile([128, 1], F32, name="rs", tag="red", bufs=4)
                nc.vector.reciprocal(rs[:npart], ssum[:npart])
                ab = ew.tile([128, S], BF16, name="ab", tag="abf", bufs=8)
                nc.vector.tensor_scalar_mul(ab[:npart], ex[:npart], rs[:npart])
                a_bf.append(ab)

            # A^T (s'-major) via PE transpose; 2 tiles covering s' 0:128, 128:160
            at = [cp.tile([128, S], F8, name=f"at{i}", tag=f"at{i}")
                  for i in range(2)]
            blocks = [  # (src tile idx, src col slice, dst tile idx, dst col off)
                (0, 0, 128, 0, 0),
                (1, 0, 128, 0, 128),
                (0, 128, S, 1, 0),
                (1, 128, S, 1, 128),
            ]
            for (sti, c0, c1, dti, dc) in blocks:
                src = a_bf[sti]
                np_src = 128 if sti == 0 else S - 128
                w_ = c1 - c0
                pt = ps.tile([128, S], BF16, name="pt", tag="pg")
                nc.tensor.transpose(
                    pt[0:w_, 0:np_src], src[0:np_src, c0:c1],
                    ident[0:np_src, 0:np_src])
                nc.vector.tensor_copy(
                    at[dti][0:w_, dc:dc + np_src], pt[0:w_, 0:np_src])

            # hf token-major [S, 512] as 2 partition tiles (transpose reads a
            # bf16 copy; fp8 PE-transpose has a stride-2 output constraint)
            hf_bf = cp.tile([128, 4, S], BF16, name="hf_bf", tag="hf_bf")
            nc.vector.tensor_copy(hf_bf[:], hf_t[:])
            hft = [cp.tile([128, 4, 128], F8, name=f"hft{i}", tag=f"hft{i}")
                   for i in range(2)]
            for k in range(4):
                pt = ps.tile([128, S], BF16, name="pt2", tag="pg")
                nc.tensor.transpose(
                    pt[0:128, 0:128], hf_bf[:, k, 0:128], ident[:])
                nc.vector.tensor_copy(hft[0][:, k, :], pt[0:128, 0:128])
                pt = ps.tile([128, S], BF16, name="pt3", tag="pg")
                nc.tensor.transpose(
                    pt[0:S - 128, 0:128], hf_bf[:, k, 128:S], ident[:])
                nc.vector.tensor_copy(
                    hft[1][0:S - 128, k, :], pt[0:S - 128, 0:128])

            # att_hist^T [512, S] = hf^T(feature-major result) : contract over s'
            att_b = cp.tile([128, 4, S], F8, name="att_b", tag="att_b")
            pa = ps.tile([128, 4, S], F32, name="pa", tag="pg",
                         padded_shape=[128, 4, 512])
            for m in range(4):
                nc.tensor.matmul(pa[:, m, :], lhsT=hft[0][:, m, :], rhs=at[0][:],
                                 start=True, stop=False)
                nc.tensor.matmul(pa[:, m, :], lhsT=hft[1][0:S - 128, m, :],
                                 rhs=at[1][0:S - 128, :],
                                 start=False, stop=True)
                nc.vector.tensor_copy(att_b[:, m, :], pa[:, m, :])

            # out = Q + tanh(att @ W2.T + b2), feature-major [512, S]
            # (W2 table is x8 so the tanh descales with scale=1/8)
            po = ps.tile([128, 4, S], F32, name="po", tag="pg",
                         padded_shape=[128, 4, 512])
            for m in range(4):
                mc = slice(m * 128, (m + 1) * 128)
                for kk in range(2):
                    nc.tensor.matmul(
                        po[:, m, :], lhsT=w2_sb[:, 2 * kk:2 * kk + 2, mc],
                        rhs=att_b[:, 2 * kk:2 * kk + 2, :],
                        start=(kk == 0), stop=(kk == 1), perf_mode=DR)
                th = ew.tile([128, S], F32, name="th", tag="ew")
                nc.scalar.activation(
                    th[:], po[:, m, :], mybir.ActivationFunctionType.Tanh,
                    bias=b2_sb[:, m:m + 1], scale=DSC)
                om = op.tile([128, S], F32, name="om", tag="om")
                nc.vector.tensor_add(om[:], th[:], qt_f[m][:])
                nc.sync.dma_start(out_d.ap()[m * 128:(m + 1) * 128, :], om[:])

    nc.compile()
    return nc


def _prep_shared(inp):
    f32 = np.float32
    emb = np.asarray(inp["emb"], f32)
    # table holds the x8 fp8-exact values in bf16 (fp8 values are exactly
    # representable in bf16, so the on-device cast is exact)
    embp = np.zeros((VOCAB, EPAD), NP_BF16)
    embp[:, :EMB] = (emb.astype(NP_BF16).astype(f32) * SC).astype(
        NP_F8).astype(NP_BF16)
    embp[0, :] = 0
    embp[:, EMB] = SC  # ones column (feature 300): x-matmul adds the bias row

    perm = _gate_perm()

    def fuse_w(wih, whh, bih, bhh):
        wx = np.zeros((EPAD, G4), f32)
        wx[0:EMB, :] = np.asarray(wih, f32).T
        wx[EMB, :] = np.asarray(bih, f32) + np.asarray(bhh, f32)
        wh = np.ascontiguousarray(np.asarray(whh, f32).T) * SC
        return (np.ascontiguousarray(wx[:, perm]).astype(NP_F8),
                np.ascontiguousarray(wh[:, perm]).astype(NP_F8))

    wqx, wqh = fuse_w(inp["Wih_q"], inp["Whh_q"], inp["bih_q"], inp["bhh_q"])
    wfx, wfh = fuse_w(inp["Wih_f"], inp["Whh_f"], inp["bih_f"], inp["bhh_f"])
    W1 = np.asarray(inp["W1"], f32)
    shared = {
        "wqx": wqx, "wqh": wqh, "wfx": wfx, "wfh": wfh,
        "w1i": np.ascontiguousarray(W1[:, :IMG].T).astype(NP_BF16),
        "w1h": np.ascontiguousarray(W1[:, IMG:].T).astype(NP_BF16),
        "b1": np.ascontiguousarray(
            np.asarray(inp["b1"], f32).reshape(4, 128).T),
        "w2": np.ascontiguousarray(
            np.asarray(inp["W2"], f32).T * SC).astype(NP_F8),
        "b2": np.ascontiguousarray(
            np.asarray(inp["b2"], f32).reshape(4, 128).T),
        "_embp": embp,
    }
    n = np.arange(S)
    mask = np.where(
        (n[:, None] // R == n[None, :] // R) & (n[None, :] % R <= n[:, None] % R),
        np.float32(0.0), np.float32(NEG))
    shared["mask"] = np.ascontiguousarray(mask.astype(f32))
    return shared


def _prep_core(inp, core, embp):
    sl = slice(core * BS, (core + 1) * BS)

    def flat(arr, L):
        # t-major flat order i = t*S + n; dma_gather reads index i from
        # [i % 16, base + i // 16], 16-partition block replicated to 128
        return np.asarray(arr[sl], np.int64).reshape(S, L).T.reshape(-1)

    qf = flat(inp["questions"], LQ)          # [3200]
    ff = flat(inp["history"], LH)            # [6400]
    uniq, inv = np.unique(np.concatenate([qf, ff]), return_inverse=True)
    assert len(uniq) <= NU_PAD
    embC = np.zeros((NU_PAD, EPAD), NP_BF16)
    embC[:len(uniq)] = embp[uniq]
    inv = inv.astype(np.int16)

    def wrap(x):
        w = x.reshape(-1, 16).T                       # [16, L*S/16]
        return np.ascontiguousarray(np.tile(w, (8, 1)))

    img = np.asarray(inp["img_features"], np.float32)[sl]          # [16, 4096]
    img_rep = np.repeat(img, R, axis=0).T                          # [4096, 160]
    return {
        "embC": embC,
        "idxq": wrap(inv[:LQ * S]),
        "idxf": wrap(inv[LQ * S:]),
        "imgrep": np.ascontiguousarray(img_rep).astype(NP_BF16),
    }


def kernel(**inputs) -> np.ndarray:
    global _STATE
    if _STATE is None:
        _STATE = _build_program()
    nc = _STATE

    shared = _prep_shared(inputs)
    embp = shared.pop("_embp")
    in_maps = []
    for c in range(N_CORES):
        m = dict(shared)
        m.update(_prep_core(inputs, c, embp))
        in_maps.append(m)

    res = run_bass_kernel_spmd(nc, in_maps, core_ids=list(range(N_CORES)))
    outs = []
    for c in range(N_CORES):
        o = np.asarray(res.results[c]["out"], np.float32)   # [512, 160]
        outs.append(o.T.reshape(BS, R, HID))
    return np.concatenate(outs, axis=0)                      # [128, 10, 512]
